# revision 42
# baseline (speedup 1.0000x reference)
"""GAT layer (nn_GATLayer_44220983279640) — Trainium2 Bass/Tile kernel.

Reference math per graph (B=16, D=512, FIN=FOUT=128, H=8):
    h  = x @ W                                         [D, F]
    s1[hd,i] = h[i] . a1[hd]   s2b[hd,j] = h[j] . a2[hd] + ab[hd]
    e  = leaky_relu(s1[:,None] + s2b[None,:])          [H, D, D]
    att = softmax_j(where(adj > 0, e, -9e15))
    out = mean_hd(att @ h)                             [D, F]

Sharding: data-parallel over batch, 2 graphs per core on 8 cores.

Key reformulation (exact math, no per-element exp/leaky_relu):
  softmax rows may be rescaled arbitrarily. With per-row scale
  exp(-(s1_i + 2)) and the identity exp(lrelu(x)) = max(exp(x), exp(0.01 x)):
      E'[j,i] = adj[j,i] * max(B_j, C_i * D_j)
      B_j = exp(s2b_j - 2)       (per-partition scalar slot, f32)
      C_i = exp(-0.99 s1_i - 2)  (row-broadcast fp16 tensor via DMA)
      D_j = exp(0.01 s2b_j)      (per-partition scalar slot, f32)
  The 4.2M-element exp/leaky_relu passes collapse into vector exps on
  [8,512] tensors in setup. Per head-graph the device does only:
    * 4x DVE tensor_scalar  (C_bcast * D_j) max B_j -> T1   (2x perf mode)
    * 1x DVE tensor_tensor  T1 * adjmask            -> E'   (2x perf mode)
    * 16 PE matmuls  E'^T-slices @ [h/8 | 1] -> U,rowsum (2 packed psum banks)
    * 2x DVE reciprocal over [128,2] rowsum column pairs
    * 4x ACT Copy(U * 1/rowsum)  psum->sbuf fp16    -> U_norm
    * 1 PE matmul    I @ U_norm accumulating over heads in PSUM
  The -2 shift keeps B <= ~9e3 (fp16-safe with 7x range margin).
  Graphs run back-to-back (b-outer): graph 1 setup and graph 0 output
  drain hide under the other graph's head loop. All C-broadcasts are
  prefetched with stride-0 DMAs from a small DRAM staging row.

Measured on trn2 (8 cores): ~72us HW exec, rel err 6.4e-4 (vs 103us for
the direct logits+Prelu+Exp formulation). Steady state paces at
~2.6us/head on the DVE chain (4 TS + 1 TT); lead-in ~20us is
DMA-bandwidth-bound (2MB masks + 2MB broadcasts). GPSIMD compute
offload was tried and abandoned: Q7 tensor ops run 5-10x below the
spec table and degrade concurrent DVE ops ~40%.
"""

from contextlib import ExitStack

import numpy as np

import concourse.bass as bass
import concourse.bacc as bacc
import concourse.tile as tile
from concourse import mybir
from concourse.bass_utils import run_bass_kernel_spmd

B, D, FIN, FOUT, H = 16, 512, 128, 128, 8
NCORES = 8
NB = B // NCORES          # graphs per core
P = 128                   # partitions
NCH = D // P              # 4 j-chunks / i-tiles
DELTA = -2.0              # global exp downshift (cancels in softmax)

F32 = mybir.dt.float32
F16 = mybir.dt.float16

# packed f32 consts (columns): W | Wa1 | Wa2 | ab_row | ones_row | delta_col
CW0, CW1 = 0, FOUT
CA1 = CW1 + H                           # Wa1 = W @ a1^T  [FIN, H]
CA2 = CA1 + H                           # Wa2 = W @ a2^T  [FIN, H]
CAB = CA2 + H                           # ab row (partition 0)  [1, H]
CDL = CAB + P                           # delta column [P, 1] = DELTA
CONST_COLS = CDL + 1

_NC_CACHE = {}


def _build_bass():
    nc = bacc.Bacc("TRN2", debug=False, num_devices=NCORES)

    xT = nc.dram_tensor("xT", [NB, FIN, D], F16, kind="ExternalInput").ap()
    adjm = nc.dram_tensor("adjm", [NB, P, NCH * D], F16, kind="ExternalInput").ap()
    consts = nc.dram_tensor("consts", [P, CONST_COLS], F32, kind="ExternalInput").ap()
    constsH = nc.dram_tensor("constsH", [P, FOUT + 2 * H], F16, kind="ExternalInput").ap()
    ident = nc.dram_tensor("ident", [P, P], F16, kind="ExternalInput").ap()
    cd = nc.dram_tensor("cd", [NB, H, D], F16).ap()
    out = nc.dram_tensor("out", [NB, D, FOUT], F32, kind="ExternalOutput").ap()

    with tile.TileContext(nc) as tc, ExitStack() as ctx:
        _kernel_body(ctx, tc, out, xT, adjm, consts, constsH, ident, cd)
    nc.compile()
    return nc


def _kernel_body(ctx, tc, out, xT, adjm, consts, constsH, ident, cd):
    nc = tc.nc
    mult, vmax = mybir.AluOpType.mult, mybir.AluOpType.max
    Copy = mybir.ActivationFunctionType.Copy
    Exp = mybir.ActivationFunctionType.Exp

    const = ctx.enter_context(tc.tile_pool(name="const", bufs=1))
    xpool = ctx.enter_context(tc.tile_pool(name="xpool", bufs=NB))
    mpool = ctx.enter_context(tc.tile_pool(name="mpool", bufs=NB))
    hpool = ctx.enter_context(tc.tile_pool(name="hpool", bufs=NB))
    bdpool = ctx.enter_context(tc.tile_pool(name="bdpool", bufs=NB))
    crow = ctx.enter_context(tc.tile_pool(name="crow", bufs=NB))
    cbpool = ctx.enter_context(tc.tile_pool(name="cbpool", bufs=NB))
    t1pool = ctx.enter_context(tc.tile_pool(name="t1pool", bufs=3))
    epool = ctx.enter_context(tc.tile_pool(name="epool", bufs=3))
    unpool = ctx.enter_context(tc.tile_pool(name="unpool", bufs=3))
    rpool = ctx.enter_context(tc.tile_pool(name="rpool", bufs=4))
    aspool = ctx.enter_context(tc.tile_pool(name="aspool", bufs=NB))
    # PSUM banks: 2 setup + 4 agg (2 packed tiles x 2 bufs) + 2 accumulators
    pset = ctx.enter_context(tc.tile_pool(name="pset", bufs=1, space="PSUM"))
    pout = ctx.enter_context(tc.tile_pool(name="pout", bufs=5, space="PSUM"))
    pacc = ctx.enter_context(tc.tile_pool(name="pacc", bufs=NB, space="PSUM"))

    cst = const.tile([P, CONST_COLS], F32)
    nc.sync.dma_start(out=cst, in_=consts)
    csth = const.tile([P, FOUT + 2 * H], F16)
    nc.sync.dma_start(out=csth, in_=constsH)
    I_sb = const.tile([P, P], F16)
    with tc.tile_wait_until(0.002):
        nc.scalar.dma_start(out=I_sb, in_=ident)
    W_sb = csth[:, CW0:CW1]
    Wa1_sb = csth[:, CW1:CA1]
    Wa2_sb = csth[:, CA1:CA2]
    ab_row = cst[0:1, CA2:CA2 + H]
    ones_row = cst[0:1, CAB:CAB + P]
    dl_col = cst[:, CDL:CDL + 1]

    G = []  # per-graph state
    for b in range(NB):
        # --- per-graph setup (DMA issues: sync=x/cd/cb, scalar=masks) -------
        x_sb = xpool.tile([FIN, D], F16, tag="x")
        nc.scalar.dma_start(out=x_sb, in_=xT[b])

        # C chain first: it gates the first tensor_scalar of the head loop.
        # C row: exp(-0.99 * s1 + DELTA), staged to DRAM, then all heads'
        # broadcasts prefetched in four stride-0 DMA slices
        p_s1 = pset.tile([P, D], F32, tag="setup")
        nc.tensor.matmul(p_s1[0:H, :], Wa1_sb, x_sb[:], start=True, stop=True)
        c_sb = crow.tile([H, D], F16, tag="Crow")
        nc.scalar.activation(
            c_sb[:], p_s1[0:H, :], Exp, scale=-0.99, bias=dl_col[0:H, :]
        )
        nc.sync.dma_start(out=cd[b], in_=c_sb[:])

        cb_all = cbpool.tile([P, H, D], F16, tag="cb")
        row0 = cd[b, 0]
        for (lo, hi), wt in (((0, 1), None), ((1, 2), 0.0045),
                             ((2, 4), 0.005), ((4, H), 0.0055)):
            with tc.tile_wait_until(wt + 0.002 * b if wt else 0.0,
                                    enable=wt is not None):
                nc.sync.dma_start(
                    out=cb_all[:, lo:hi, :],
                    in_=bass.AP(
                        tensor=cd.tensor, offset=row0.offset + lo * D,
                        ap=[[0, P], [D, hi - lo], row0.ap[-1]],
                    ),
                )

        # B/D per-partition scalars: s2bT[j, hd] = (x^T Wa2 + ab)[j, hd]
        B_col, D_col = [], []
        for c in range(NCH):
            p_s = pset.tile([P, D], F32, tag="setup")
            nc.tensor.matmul(
                p_s[:, 0:H], x_sb[:, bass.ts(c, P)], Wa2_sb, start=True, stop=False
            )
            nc.tensor.matmul(p_s[:, 0:H], ones_row, ab_row, start=False, stop=True)
            bc = bdpool.tile([P, H], F32, tag=f"B{c}")
            nc.scalar.activation(bc[:], p_s[:, 0:H], Exp, bias=dl_col)
            dc = bdpool.tile([P, H], F32, tag=f"D{c}")
            nc.scalar.activation(dc[:], p_s[:, 0:H], Exp, scale=0.01)
            B_col.append(bc)
            D_col.append(dc)

        # h tiles + ones column, fp16, h pre-scaled by 1/H
        haug = []
        for c in range(NCH):
            p_h = pset.tile([P, D], F32, tag="setup")
            nc.tensor.matmul(
                p_h[:, 0:FOUT], x_sb[:, bass.ts(c, P)], W_sb, start=True, stop=True
            )
            ha = hpool.tile([P, FOUT + 1], F16, tag=f"haug{c}")
            nc.scalar.activation(ha[:, 0:FOUT], p_h[:, 0:FOUT], Copy, scale=1.0 / H)
            nc.vector.memset(ha[:, FOUT:FOUT + 1], 1.0)
            haug.append(ha)

        # masks issued last so they don't delay x/cd/cb on the DMA engines
        m_sb = mpool.tile([P, NCH * D], F16, tag="m")
        with tc.tile_wait_until(0.005 + 0.002 * b):
            nc.sync.dma_start(out=m_sb, in_=adjm[b])

        p_acc = pacc.tile([P, NCH * FOUT], F32, tag="acc")
        G.append(dict(
            m_sb=m_sb, haug=haug, B_col=B_col, D_col=D_col,
            cb_all=cb_all, p_acc=p_acc,
        ))

    # --- main head loops, one graph at a time --------------------------------
    for b in range(NB):
        for hd in range(H):
            g = G[b]
            m_sb, haug, cb_all = g["m_sb"], g["haug"], g["cb_all"]
            cb = cb_all[:, hd, :]
            # T1 = (C_i * D_j) max B_j
            t1 = t1pool.tile([P, NCH * D], F16, tag="t1")
            for c in range(NCH):
                nc.vector.tensor_scalar(
                    out=t1[:, bass.ts(c, D)], in0=cb,
                    scalar1=g["D_col"][c][:, hd:hd + 1],
                    scalar2=g["B_col"][c][:, hd:hd + 1],
                    op0=mult, op1=vmax,
                )
            # E' = T1 * adjmask (one 2x-mode tensor_tensor)
            E = epool.tile([P, NCH * D], F16, tag="E")
            nc.vector.tensor_tensor(out=E[:], in0=t1[:], in1=m_sb[:], op=mult)

            # agg: psum[i-tile t] += E'^T-slice @ [h/8 | 1]; two [128,129]
            # tiles share each PSUM bank so rowsums batch into 2 reciprocals
            po2 = [pout.tile([P, 2, FOUT + 1], F32, tag="po", name=f"po{k}")
                   for k in range(2)]
            p_os = [po2[t // 2][:, t % 2, :] for t in range(NCH)]
            for t in range(NCH):
                for c in range(NCH):
                    nc.tensor.matmul(
                        p_os[t],
                        E[:, c * D + t * P: c * D + (t + 1) * P],
                        haug[c][:],
                        start=(c == 0),
                        stop=(c == NCH - 1),
                    )

            rall = rpool.tile([P, NCH], F32, tag="r")
            for k in range(2):
                nc.vector.reciprocal(
                    rall[:, 2 * k:2 * k + 2], po2[k][:, :, FOUT]
                )

            # normalize per head on ACT (scale = 1/rowsum), accumulate on PE
            un = unpool.tile([P, NCH * FOUT], F16, tag="un")
            for t in range(NCH):
                nc.scalar.activation(
                    un[:, bass.ts(t, FOUT)], p_os[t][:, 0:FOUT], Copy,
                    scale=rall[:, t:t + 1],
                )
            nc.tensor.matmul(
                g["p_acc"][:], I_sb, un[:], start=(hd == 0), stop=(hd == H - 1)
            )

    for b in range(NB):
        acc_sb = aspool.tile([P, NCH, FOUT], F32, tag="accsb")
        nc.scalar.activation(acc_sb[:], G[b]["p_acc"][:], Copy)
        ob = out[b]
        nc.sync.dma_start(
            out=bass.AP(
                tensor=out.tensor, offset=ob.offset,
                ap=[[FOUT, P], [P * FOUT, NCH], [1, FOUT]],
            ),
            in_=acc_sb[:],
        )


def _prep_core_inputs(input, adj, W, a_w, a_b, core):
    gs = slice(core * NB, (core + 1) * NB)
    x_c = np.asarray(input[gs], dtype=np.float32)     # [NB, D, FIN]
    adj_c = np.asarray(adj[gs])                       # [NB, D, D] int32
    xT = np.ascontiguousarray(x_c.transpose(0, 2, 1)).astype(np.float16)
    adjT = (adj_c.transpose(0, 2, 1) > 0)             # [NB, j, i]
    # [NB, j, i] -> [NB, p, c, i]  (j = c*128 + p)
    adjm = np.ascontiguousarray(
        adjT.reshape(NB, NCH, P, D).transpose(0, 2, 1, 3)
        .reshape(NB, P, NCH * D).astype(np.float16)
    )
    return {
        "xT": xT,
        "adjm": adjm,
        "consts": _pack_consts(W, a_w, a_b),
        "constsH": _pack_consts_h(W, a_w),
        "ident": np.eye(P, dtype=np.float16),
    }


def _pack_consts_h(W, a_w):
    W = np.asarray(W, dtype=np.float32)
    a_w = np.asarray(a_w, dtype=np.float32)
    c = np.zeros((P, FOUT + 2 * H), dtype=np.float32)
    c[:, CW0:CW1] = W
    c[:, CW1:CA1] = W @ a_w[:, :FOUT].T               # Wa1 [FIN, H]
    c[:, CA1:CA2] = W @ a_w[:, FOUT:].T               # Wa2 [FIN, H]
    return c.astype(np.float16)


def _pack_consts(W, a_w, a_b):
    W = np.asarray(W, dtype=np.float32)
    a_w = np.asarray(a_w, dtype=np.float32)
    a_b = np.asarray(a_b, dtype=np.float32)
    c = np.zeros((P, CONST_COLS), dtype=np.float32)
    c[:, CW0:CW1] = W
    c[:, CW1:CA1] = W @ a_w[:, :FOUT].T               # Wa1 [FIN, H]
    c[:, CA1:CA2] = W @ a_w[:, FOUT:].T               # Wa2 [FIN, H]
    c[0, CA2:CA2 + H] = a_b
    c[0, CAB:CAB + P] = 1.0
    c[:, CDL] = DELTA
    return c


def get_nc():
    if "nc" not in _NC_CACHE:
        _NC_CACHE["nc"] = _build_bass()
    return _NC_CACHE["nc"]


def run_on_device(in_maps, **kwargs):
    return run_bass_kernel_spmd(get_nc(), in_maps, list(range(NCORES)), **kwargs)


def kernel(input, adj, W, a_w, a_b):
    input = np.asarray(input, dtype=np.float32)
    adj = np.asarray(adj)

    in_maps = [
        _prep_core_inputs(input, adj, W, a_w, a_b, c) for c in range(NCORES)
    ]
    res = run_on_device(in_maps)
    outs = [res.results[c]["out"] for c in range(NCORES)]
    return np.concatenate(outs, axis=0).astype(np.float32)


if __name__ == "__main__":
    nc = get_nc()
    print("built ok")


# revision 43
# speedup vs baseline: 1.0707x; 1.0707x over previous
"""GAT layer (nn_GATLayer_44220983279640) — Trainium2 Bass/Tile kernel.

Reference math per graph (B=16, D=512, FIN=FOUT=128, H=8):
    h  = x @ W                                         [D, F]
    s1[hd,i] = h[i] . a1[hd]   s2b[hd,j] = h[j] . a2[hd] + ab[hd]
    e  = leaky_relu(s1[:,None] + s2b[None,:])          [H, D, D]
    att = softmax_j(where(adj > 0, e, -9e15))
    out = mean_hd(att @ h)                             [D, F]

Sharding: data-parallel over batch, 2 graphs per core on 8 cores.

Key reformulation (exact math, no per-element exp/leaky_relu):
  softmax rows may be rescaled arbitrarily. With per-row scale
  exp(-(s1_i + 2)) and the identity exp(lrelu(x)) = max(exp(x), exp(0.01 x)):
      E'[j,i] = adj[j,i] * max(B_j, C_i * D_j)
      B_j = exp(s2b_j - 2)       (per-partition scalar slot, f32)
      C_i = exp(-0.99 s1_i - 2)  (row-broadcast fp16 tensor via DMA)
      D_j = exp(0.01 s2b_j)      (per-partition scalar slot, f32)
  The 4.2M-element exp/leaky_relu passes collapse into vector exps on
  [8,512] tensors in setup. Per head-graph the device does only:
    * 4x DVE tensor_scalar  (C_bcast * D_j) max B_j -> T1   (2x perf mode)
    * 1x DVE tensor_tensor  T1 * adjmask            -> E'   (2x perf mode)
    * 16 PE matmuls  E'^T-slices @ [h/8 | 1] -> U,rowsum (2 packed psum banks)
    * 2x DVE reciprocal over [128,2] rowsum column pairs
    * 4x ACT Copy(U * 1/rowsum)  psum->sbuf fp16    -> U_norm
    * 1 PE matmul    I @ U_norm accumulating over heads in PSUM
  The -2 shift keeps B <= ~9e3 (fp16-safe with 7x range margin).
  Graphs run back-to-back (b-outer): graph 1 setup and graph 0 output
  drain hide under the other graph's head loop. All C-broadcasts are
  prefetched with stride-0 DMAs from a small DRAM staging row.

Measured on trn2 (8 cores): ~72us HW exec, rel err 6.4e-4 (vs 103us for
the direct logits+Prelu+Exp formulation). Steady state paces at
~2.6us/head on the DVE chain (4 TS + 1 TT); lead-in ~20us is
DMA-bandwidth-bound (2MB masks + 2MB broadcasts). GPSIMD compute
offload was tried and abandoned: Q7 tensor ops run 5-10x below the
spec table and degrade concurrent DVE ops ~40%.
"""

from contextlib import ExitStack

import numpy as np

import concourse.bass as bass
import concourse.bacc as bacc
import concourse.tile as tile
from concourse import mybir
from concourse.bass_utils import run_bass_kernel_spmd

B, D, FIN, FOUT, H = 16, 512, 128, 128, 8
NCORES = 8
NB = B // NCORES          # graphs per core
P = 128                   # partitions
NCH = D // P              # 4 j-chunks / i-tiles
DELTA = -2.0              # global exp downshift (cancels in softmax)

F32 = mybir.dt.float32
F16 = mybir.dt.float16

# packed f32 consts (columns): W | Wa1 | Wa2 | ab_row | ones_row | delta_col
CW0, CW1 = 0, FOUT
CA1 = CW1 + H                           # Wa1 = W @ a1^T  [FIN, H]
CA2 = CA1 + H                           # Wa2 = W @ a2^T  [FIN, H]
CAB = CA2 + H                           # ab row (partition 0)  [1, H]
CDL = CAB + P                           # delta column [P, 1] = DELTA
CONST_COLS = CDL + 1

_NC_CACHE = {}


def _build_bass():
    nc = bacc.Bacc("TRN2", debug=False, num_devices=NCORES)

    xT = nc.dram_tensor("xT", [NB, FIN, D], F16, kind="ExternalInput").ap()
    adjm = nc.dram_tensor("adjm", [NB, P, NCH * D], F16, kind="ExternalInput").ap()
    consts = nc.dram_tensor("consts", [P, CONST_COLS], F32, kind="ExternalInput").ap()
    constsH = nc.dram_tensor("constsH", [P, FOUT + 2 * H], F16, kind="ExternalInput").ap()
    ident = nc.dram_tensor("ident", [P, P], F16, kind="ExternalInput").ap()
    cd = nc.dram_tensor("cd", [NB, H, D], F16).ap()
    out = nc.dram_tensor("out", [NB, D, FOUT], F32, kind="ExternalOutput").ap()

    with tile.TileContext(nc) as tc, ExitStack() as ctx:
        _kernel_body(ctx, tc, out, xT, adjm, consts, constsH, ident, cd)
    nc.compile()
    return nc


def _kernel_body(ctx, tc, out, xT, adjm, consts, constsH, ident, cd):
    nc = tc.nc
    mult, vmax = mybir.AluOpType.mult, mybir.AluOpType.max
    Copy = mybir.ActivationFunctionType.Copy
    Exp = mybir.ActivationFunctionType.Exp

    const = ctx.enter_context(tc.tile_pool(name="const", bufs=1))
    xpool = ctx.enter_context(tc.tile_pool(name="xpool", bufs=NB))
    mpool = ctx.enter_context(tc.tile_pool(name="mpool", bufs=NB))
    hpool = ctx.enter_context(tc.tile_pool(name="hpool", bufs=NB))
    bdpool = ctx.enter_context(tc.tile_pool(name="bdpool", bufs=NB))
    crow = ctx.enter_context(tc.tile_pool(name="crow", bufs=NB))
    cbpool = ctx.enter_context(tc.tile_pool(name="cbpool", bufs=NB))
    t1pool = ctx.enter_context(tc.tile_pool(name="t1pool", bufs=3))
    epool = ctx.enter_context(tc.tile_pool(name="epool", bufs=3))
    unpool = ctx.enter_context(tc.tile_pool(name="unpool", bufs=3))
    rpool = ctx.enter_context(tc.tile_pool(name="rpool", bufs=4))
    aspool = ctx.enter_context(tc.tile_pool(name="aspool", bufs=NB))
    # PSUM banks: 2 setup + 4 agg (2 packed tiles x 2 bufs) + 2 accumulators
    pset = ctx.enter_context(tc.tile_pool(name="pset", bufs=2, space="PSUM"))
    pout = ctx.enter_context(tc.tile_pool(name="pout", bufs=4, space="PSUM"))
    pacc = ctx.enter_context(tc.tile_pool(name="pacc", bufs=NB, space="PSUM"))

    cst = const.tile([P, CONST_COLS], F32)
    nc.sync.dma_start(out=cst, in_=consts)
    csth = const.tile([P, FOUT + 2 * H], F16)
    nc.sync.dma_start(out=csth, in_=constsH)
    I_sb = const.tile([P, P], F16)
    with tc.tile_wait_until(0.002):
        nc.scalar.dma_start(out=I_sb, in_=ident)
    W_sb = csth[:, CW0:CW1]
    Wa1_sb = csth[:, CW1:CA1]
    Wa2_sb = csth[:, CA1:CA2]
    ab_row = cst[0:1, CA2:CA2 + H]
    ones_row = cst[0:1, CAB:CAB + P]
    dl_col = cst[:, CDL:CDL + 1]

    G = []  # per-graph state
    for b in range(NB):
        # --- per-graph setup (DMA issues: sync=x/cd/cb, scalar=masks) -------
        x_sb = xpool.tile([FIN, D], F16, tag="x")
        nc.scalar.dma_start(out=x_sb, in_=xT[b])

        # C chain first: it gates the first tensor_scalar of the head loop.
        # C row: exp(-0.99 * s1 + DELTA), staged to DRAM, then all heads'
        # broadcasts prefetched in four stride-0 DMA slices
        p_s1 = pset.tile([P, D], F32, tag="setup")
        nc.tensor.matmul(p_s1[0:H, :], Wa1_sb, x_sb[:], start=True, stop=True)
        c_sb = crow.tile([H, D], F16, tag="Crow")
        nc.scalar.activation(
            c_sb[:], p_s1[0:H, :], Exp, scale=-0.99, bias=dl_col[0:H, :]
        )
        nc.sync.dma_start(out=cd[b], in_=c_sb[:])

        cb_all = cbpool.tile([P, H, D], F16, tag="cb")
        row0 = cd[b, 0]
        for (lo, hi), wt in (((0, 1), None), ((1, 2), 0.0045),
                             ((2, 4), 0.005), ((4, H), 0.0055)):
            with tc.tile_wait_until(wt + 0.002 * b if wt else 0.0,
                                    enable=wt is not None):
                nc.sync.dma_start(
                    out=cb_all[:, lo:hi, :],
                    in_=bass.AP(
                        tensor=cd.tensor, offset=row0.offset + lo * D,
                        ap=[[0, P], [D, hi - lo], row0.ap[-1]],
                    ),
                )

        # B/D per-partition scalars: s2bT[j, hd] = (x^T Wa2 + ab)[j, hd]
        B_col, D_col = [], []
        for c in range(NCH):
            p_s = pset.tile([P, D], F32, tag="setup")
            nc.tensor.matmul(
                p_s[:, 0:H], x_sb[:, bass.ts(c, P)], Wa2_sb, start=True, stop=False
            )
            nc.tensor.matmul(p_s[:, 0:H], ones_row, ab_row, start=False, stop=True)
            bc = bdpool.tile([P, H], F32, tag=f"B{c}")
            nc.scalar.activation(bc[:], p_s[:, 0:H], Exp, bias=dl_col)
            dc = bdpool.tile([P, H], F32, tag=f"D{c}")
            nc.scalar.activation(dc[:], p_s[:, 0:H], Exp, scale=0.01)
            B_col.append(bc)
            D_col.append(dc)

        # h tiles + ones column, fp16, h pre-scaled by 1/H
        haug = []
        for c in range(NCH):
            p_h = pset.tile([P, D], F32, tag="setup")
            nc.tensor.matmul(
                p_h[:, 0:FOUT], x_sb[:, bass.ts(c, P)], W_sb, start=True, stop=True
            )
            ha = hpool.tile([P, FOUT + 1], F16, tag=f"haug{c}")
            nc.scalar.activation(ha[:, 0:FOUT], p_h[:, 0:FOUT], Copy, scale=1.0 / H)
            nc.vector.memset(ha[:, FOUT:FOUT + 1], 1.0)
            haug.append(ha)

        # masks issued last so they don't delay x/cd/cb on the DMA engines
        m_sb = mpool.tile([P, NCH * D], F16, tag="m")
        with tc.tile_wait_until(0.005 + 0.002 * b):
            nc.sync.dma_start(out=m_sb, in_=adjm[b])

        p_acc = pacc.tile([P, NCH * FOUT], F32, tag="acc")
        G.append(dict(
            m_sb=m_sb, haug=haug, B_col=B_col, D_col=D_col,
            cb_all=cb_all, p_acc=p_acc,
        ))

    # --- main head loops, one graph at a time --------------------------------
    for b in range(NB):
        for hd in range(H):
            g = G[b]
            m_sb, haug, cb_all = g["m_sb"], g["haug"], g["cb_all"]
            cb = cb_all[:, hd, :]
            # T1 = (C_i * D_j) max B_j
            t1 = t1pool.tile([P, NCH * D], F16, tag="t1")
            for c in range(NCH):
                nc.vector.tensor_scalar(
                    out=t1[:, bass.ts(c, D)], in0=cb,
                    scalar1=g["D_col"][c][:, hd:hd + 1],
                    scalar2=g["B_col"][c][:, hd:hd + 1],
                    op0=mult, op1=vmax,
                )
            # E' = T1 * adjmask (one 2x-mode tensor_tensor)
            E = epool.tile([P, NCH * D], F16, tag="E")
            nc.vector.tensor_tensor(out=E[:], in0=t1[:], in1=m_sb[:], op=mult)

            # agg: psum[i-tile t] += E'^T-slice @ [h/8 | 1]; two [128,129]
            # tiles share each PSUM bank so rowsums batch into 2 reciprocals
            po2 = [pout.tile([P, 2, FOUT + 1], F32, tag="po", name=f"po{k}")
                   for k in range(2)]
            p_os = [po2[t // 2][:, t % 2, :] for t in range(NCH)]
            for t in range(NCH):
                for c in range(NCH):
                    nc.tensor.matmul(
                        p_os[t],
                        E[:, c * D + t * P: c * D + (t + 1) * P],
                        haug[c][:],
                        start=(c == 0),
                        stop=(c == NCH - 1),
                    )

            rall = rpool.tile([P, NCH], F32, tag="r")
            for k in range(2):
                nc.vector.reciprocal(
                    rall[:, 2 * k:2 * k + 2], po2[k][:, :, FOUT]
                )

            # normalize per head on ACT (scale = 1/rowsum), accumulate on PE
            un = unpool.tile([P, NCH * FOUT], F16, tag="un")
            for t in range(NCH):
                nc.scalar.activation(
                    un[:, bass.ts(t, FOUT)], p_os[t][:, 0:FOUT], Copy,
                    scale=rall[:, t:t + 1],
                )
            nc.tensor.matmul(
                g["p_acc"][:], I_sb, un[:], start=(hd == 0), stop=(hd == H - 1)
            )

    for b in range(NB):
        acc_sb = aspool.tile([P, NCH, FOUT], F32, tag="accsb")
        nc.scalar.activation(acc_sb[:], G[b]["p_acc"][:], Copy)
        ob = out[b]
        nc.sync.dma_start(
            out=bass.AP(
                tensor=out.tensor, offset=ob.offset,
                ap=[[FOUT, P], [P * FOUT, NCH], [1, FOUT]],
            ),
            in_=acc_sb[:],
        )


def _prep_core_inputs(input, adj, W, a_w, a_b, core):
    gs = slice(core * NB, (core + 1) * NB)
    x_c = np.asarray(input[gs], dtype=np.float32)     # [NB, D, FIN]
    adj_c = np.asarray(adj[gs])                       # [NB, D, D] int32
    xT = np.ascontiguousarray(x_c.transpose(0, 2, 1)).astype(np.float16)
    adjT = (adj_c.transpose(0, 2, 1) > 0)             # [NB, j, i]
    # [NB, j, i] -> [NB, p, c, i]  (j = c*128 + p)
    adjm = np.ascontiguousarray(
        adjT.reshape(NB, NCH, P, D).transpose(0, 2, 1, 3)
        .reshape(NB, P, NCH * D).astype(np.float16)
    )
    return {
        "xT": xT,
        "adjm": adjm,
        "consts": _pack_consts(W, a_w, a_b),
        "constsH": _pack_consts_h(W, a_w),
        "ident": np.eye(P, dtype=np.float16),
    }


def _pack_consts_h(W, a_w):
    W = np.asarray(W, dtype=np.float32)
    a_w = np.asarray(a_w, dtype=np.float32)
    c = np.zeros((P, FOUT + 2 * H), dtype=np.float32)
    c[:, CW0:CW1] = W
    c[:, CW1:CA1] = W @ a_w[:, :FOUT].T               # Wa1 [FIN, H]
    c[:, CA1:CA2] = W @ a_w[:, FOUT:].T               # Wa2 [FIN, H]
    return c.astype(np.float16)


def _pack_consts(W, a_w, a_b):
    W = np.asarray(W, dtype=np.float32)
    a_w = np.asarray(a_w, dtype=np.float32)
    a_b = np.asarray(a_b, dtype=np.float32)
    c = np.zeros((P, CONST_COLS), dtype=np.float32)
    c[:, CW0:CW1] = W
    c[:, CW1:CA1] = W @ a_w[:, :FOUT].T               # Wa1 [FIN, H]
    c[:, CA1:CA2] = W @ a_w[:, FOUT:].T               # Wa2 [FIN, H]
    c[0, CA2:CA2 + H] = a_b
    c[0, CAB:CAB + P] = 1.0
    c[:, CDL] = DELTA
    return c


def get_nc():
    if "nc" not in _NC_CACHE:
        _NC_CACHE["nc"] = _build_bass()
    return _NC_CACHE["nc"]


def run_on_device(in_maps, **kwargs):
    return run_bass_kernel_spmd(get_nc(), in_maps, list(range(NCORES)), **kwargs)


def kernel(input, adj, W, a_w, a_b):
    input = np.asarray(input, dtype=np.float32)
    adj = np.asarray(adj)

    in_maps = [
        _prep_core_inputs(input, adj, W, a_w, a_b, c) for c in range(NCORES)
    ]
    res = run_on_device(in_maps)
    outs = [res.results[c]["out"] for c in range(NCORES)]
    return np.concatenate(outs, axis=0).astype(np.float32)


if __name__ == "__main__":
    nc = get_nc()
    print("built ok")
